# revision 30
# baseline (speedup 1.0000x reference)
import sys

for _p in ("/opt/trn_rl_repo", "/root/.axon_site/_ro/trn_rl_repo"):
    if _p not in sys.path:
        sys.path.append(_p)

import numpy as np

# Problem: B=8 batches of cross-attention-like softmax matmul, one batch per core.
#   S[e,t] = sum_d enc[e,d] * dec[t,d]
#   A = softmax(S, axis=t)
#   C[t,d] = sum_e A[e,t] * enc[e,d]
#
# Host uploads f16 operands pre-transposed (d-major) so the PE does zero
# transposes; phase C runs half the e-contraction in fp8 DoubleRow (2x rate).
B, S, D = 8, 2048, 1024
P = 128
EB = S // P   # 16 e-blocks
TC = S // 512 # 4 t-chunks of 512 (matmul free-dim limit)
DC = D // P   # 8 d-chunks (contraction for scores)
N8 = 4        # fp8 e-block PAIRS in phase C (e-blocks 0..2*N8-1 are fp8)

_NC_CACHE = None


def _blockT(xf):
    # [S, D] f16 -> [DC*TC*P, 512] where block (di, j) = x.T[di*128:+128,
    # j*512:+512] stored contiguously (one 128KB DMA unit per block)
    return np.ascontiguousarray(
        xf.T.reshape(DC, P, TC, 512).transpose(0, 2, 1, 3).reshape(DC * TC * P, 512)
    )


def make_in_maps(enc_outputs, dec_outputs):
    maps = []
    for b in range(B):
        ef = np.asarray(enc_outputs[b], dtype=np.float32).astype(np.float16)
        df = np.asarray(dec_outputs[b], dtype=np.float32).astype(np.float16)
        maps.append(
            {
                "enc_nat": ef,
                "encT": _blockT(ef),
                "decT": _blockT(df),
            }
        )
    return maps


def _build():
    import concourse.bacc as bacc
    import concourse.tile as tile
    from concourse import mybir

    F32 = mybir.dt.float32
    F16 = mybir.dt.float16
    F8 = mybir.dt.float8e4
    DR = mybir.MatmulPerfMode.DoubleRow

    nc = bacc.Bacc("TRN2", target_bir_lowering=False, debug=False, num_devices=B)
    encN = nc.declare_dram_parameter("enc_nat", [S, D], F16, isOutput=False)
    encT = nc.declare_dram_parameter("encT", [DC * TC * P, 512], F16, isOutput=False)
    decT = nc.declare_dram_parameter("decT", [DC * TC * P, 512], F16, isOutput=False)
    out = nc.declare_dram_parameter("out", [S, D], F32, isOutput=True)

    with tile.TileContext(nc) as tc:
        with (
            tc.tile_pool(name="const", bufs=1) as const_pool,
            tc.tile_pool(name="bigT", bufs=1) as bigT_pool,
            tc.tile_pool(name="encn", bufs=1) as encn_pool,
            tc.tile_pool(name="pmat", bufs=1) as p_pool,
            tc.tile_pool(name="fp8", bufs=1) as f8_pool,
            tc.tile_pool(name="stats", bufs=6) as stats_pool,
            tc.tile_pool(name="ostage", bufs=2) as out_pool,
            tc.tile_pool(name="psum_s", bufs=1, space="PSUM") as psum_s,
        ):
            # warm-up operand: memset lands ~2us before any DMA (DVE's
            # framework preamble ends first), so the PE clock ramp starts
            # earlier
            ident = const_pool.tile([P, P], F16, name="ident")
            nc.vector.memset(ident[:], 1.0)

            # d-major transposed operands (uploaded pre-transposed):
            # encTbig[:, d*S + s] = enc[s, d*P + dd]  (dd = partition)
            encTbig = bigT_pool.tile([P, DC * S], F16, name="encTbig")
            decTbig = bigT_pool.tile([P, DC * S], F16, name="decTbig")
            encn = [encn_pool.tile([P, D], F16, name=f"encn{e}") for e in range(EB)]
            pmat = [p_pool.tile([P, S], F16, name=f"p{e}") for e in range(EB)]
            pm8 = [f8_pool.tile([P, 2 * S], F8, name=f"pm8_{p_}") for p_ in range(N8)]
            ez8 = [f8_pool.tile([P, 2 * D], F8, name=f"ez8_{p_}") for p_ in range(N8)]

            # ---- input DMA streams ----------------------------------------
            # Blocked DRAM layouts: unit (di, q) of encT/decT is one
            # contiguous 128KB read (big DMA packets). decT is split across
            # both HWDGE queues by di parity so each j-layer lands fast;
            # gpsimd's SWDGE carries encT; enc natural trails (not needed
            # until softmax_eb / phase C).
            def blk(t, di, q):
                r = (di * TC + q) * P
                return t[r : r + P, :]

            def dec_unit(eng, di, j):
                eng.dma_start(
                    out=decTbig[:, di * S + j * 512 : di * S + (j + 1) * 512],
                    in_=blk(decT, di, j),
                )

            def enc_unit(eng, di, eq):
                eng.dma_start(
                    out=encTbig[:, di * S + eq * 512 : di * S + (eq + 1) * 512],
                    in_=blk(encT, di, eq),
                )

            def encn_dma(e):
                # scalar-queue dispatch, interleaved into phase A emission so
                # the exps (same engine) are never stuck behind a wall of
                # ring-credit-limited DMA dispatches
                nc.scalar.dma_start(out=encn[e][:], in_=encN[e * P : (e + 1) * P, :])

            # first layers split 3 ways (scalar is idle this early); the
            # rest stream on sync (decT) and gpsimd (encT). j1's second half
            # also rides scalar so the j1 wall arrives ~4us sooner.
            for di in range(0, 4):
                dec_unit(nc.sync, di, 0)
                enc_unit(nc.gpsimd, di, 0)
            for di in range(4, DC):
                enc_unit(nc.scalar, di, 0)
                dec_unit(nc.scalar, di, 0)
            for di in range(0, 4):
                dec_unit(nc.sync, di, 1)
            for di in range(4, DC):
                dec_unit(nc.scalar, di, 1)
            encn_dma(0)
            encn_dma(1)
            for j in range(2, TC):
                for di in range(DC):
                    dec_unit(nc.sync, di, j)
            for eq in range(1, 4):
                for di in range(DC):
                    enc_unit(nc.gpsimd, di, eq)

            # PE clock warm-up during the DMA preamble
            warm = psum_s.tile([P, 512], F32, tag="tp", bufs=2, name="warm")
            for _ in range(16):
                nc.tensor.matmul(
                    warm[:, 0:P], lhsT=ident, rhs=ident, start=True, stop=True
                )

            # ---- phase A: scores + online softmax -------------------------
            eb_state = {}

            def get_st(e):
                return eb_state.setdefault(
                    e,
                    {
                        "negm": [None] * TC,  # running NEGATED max per chunk
                        "zparts": stats_pool.tile([P, TC], F32, name=f"zp{e}", bufs=16),
                    },
                )

            def chunk_stats(e, j, sch):
                # online-softmax: exp against the RUNNING max so the chunk's
                # PSUM tile releases immediately (in allocation order, which
                # the round-robin slot allocator requires for pipelining);
                # earlier pmat slices get rescaled at eb end. The max chain is
                # kept negated (reduce negate=True) so the exp bias needs no
                # extra negation op.
                st = eb_state[e]
                nmj = stats_pool.tile([P, 1], F32, name="nmj", bufs=48)
                nc.vector.reduce_max(
                    out=nmj, in_=sch[:], axis=mybir.AxisListType.X, negate=True
                )
                if j == 0:
                    nm_run = nmj
                else:
                    nm_run = stats_pool.tile([P, 1], F32, name="nmrun", bufs=64)
                    nc.vector.tensor_tensor(
                        out=nm_run, in0=st["negm"][j - 1], in1=nmj,
                        op=mybir.AluOpType.min,
                    )
                st["negm"][j] = nm_run
                nc.scalar.activation(
                    out=pmat[e][:, j * 512 : (j + 1) * 512],
                    in_=sch[:],
                    func=mybir.ActivationFunctionType.Exp,
                    bias=nm_run,
                    scale=1.0,
                    accum_out=st["zparts"][:, j : j + 1],
                )

            def mm_chunk(e, j):
                get_st(e)
                sch = psum_s.tile([P, 512], F32, tag="sps", bufs=6, name=f"s{e}_{j}")
                for di in range(DC):
                    nc.tensor.matmul(
                        sch[:],
                        lhsT=encTbig[:, di * S + e * P : di * S + (e + 1) * P],
                        rhs=decTbig[:, di * S + j * 512 : di * S + (j + 1) * 512],
                        start=(di == 0),
                        stop=(di == DC - 1),
                    )
                chunk_stats(e, j, sch)

            def softmax_eb(e, on_scalar=False):
                st = eb_state[e]
                negm3 = st["negm"][TC - 1]
                fp8b = e < 2 * N8
                pr, half = e // 2, e % 2
                for j in range(TC - 1):
                    # cj = exp(m_j - m3) = exp(-negm_j + negm3) on scalar; the
                    # rescale (DVE, or scalar for a deferred burst) also
                    # refreshes zparts via accum_out and, for fp8 blocks,
                    # writes the fp8 copy directly.
                    cj = stats_pool.tile([P, 1], F32, name="cj", bufs=8)
                    nc.scalar.activation(
                        out=cj, in_=st["negm"][j],
                        func=mybir.ActivationFunctionType.Exp,
                        bias=negm3, scale=-1.0,
                    )
                    if fp8b:
                        tgt = pm8[pr][:, half * S + j * 512 : half * S + (j + 1) * 512]
                    else:
                        tgt = pmat[e][:, j * 512 : (j + 1) * 512]
                    if on_scalar:
                        nc.scalar.activation(
                            out=tgt,
                            in_=pmat[e][:, j * 512 : (j + 1) * 512],
                            func=mybir.ActivationFunctionType.Copy,
                            scale=cj,
                            accum_out=st["zparts"][:, j : j + 1],
                        )
                    else:
                        nc.vector.tensor_scalar(
                            out=tgt,
                            in0=pmat[e][:, j * 512 : (j + 1) * 512],
                            scalar1=cj,
                            scalar2=None,
                            op0=mybir.AluOpType.mult,
                            op1=mybir.AluOpType.add,
                            accum_out=st["zparts"][:, j : j + 1],
                        )
                if fp8b:
                    j = TC - 1
                    nc.scalar.activation(
                        out=pm8[pr][:, half * S + j * 512 : half * S + (j + 1) * 512],
                        in_=pmat[e][:, j * 512 : (j + 1) * 512],
                        func=mybir.ActivationFunctionType.Copy,
                    )
                z = stats_pool.tile([P, 1], F32, name="z")
                nc.vector.reduce_sum(out=z, in_=st["zparts"][:], axis=mybir.AxisListType.X)
                zinv = stats_pool.tile([P, 1], F32, name="zinv")
                nc.vector.reciprocal(zinv, z)
                if fp8b:
                    nc.vector.tensor_scalar_mul(
                        ez8[pr][:, half * D : (half + 1) * D], encn[e][:], zinv
                    )
                else:
                    nc.vector.tensor_scalar_mul(encn[e][:], encn[e][:], zinv)

            # emission order: greedy, e-major. Complete low e-blocks ASAP so
            # the softmax_eb bursts (DVE/scalar work + fp8 copies) spread
            # across phase A instead of clustering at its end. Arrival times
            # (us) modeled from the measured DMA queue cadence.
            t_dec = {0: 16.0, 1: 21.0, 2: 28.0, 3: 36.0}
            t_enc = {0: 16.0, 1: 23.0, 2: 33.0, 3: 43.0}
            pe_t = 13.0
            CH = 1.8
            next_j = [0] * EB
            seq = []
            while len(seq) < EB * TC:
                pick = None
                for e in range(EB):
                    j = next_j[e]
                    if j >= TC:
                        continue
                    if max(t_enc[e // 4], t_dec[j]) <= pe_t:
                        pick = (e, j)
                        break
                if pick is None:
                    r, e = min(
                        (max(t_enc[e // 4], t_dec[next_j[e]]), e)
                        for e in range(EB)
                        if next_j[e] < TC
                    )
                    pick, pe_t = (e, next_j[e]), r
                pe_t += CH
                seq.append(pick)
                next_j[pick[0]] += 1
            finish_order = [e for e, j in seq if j == TC - 1]
            # the last TWO e-blocks to finish get their softmax bursts
            # deferred into phase C's first group, where leading matmuls
            # cover their serial chains
            defer = finish_order[-2:]
            for e, j in seq:
                mm_chunk(e, j)
                if j == TC - 1:
                    if e not in defer:
                        softmax_eb(e)
                    if e + 2 < EB:
                        encn_dma(e + 2)

            # ---- Phase C: C[t,:] = sum_e P[e,t] * encZ[e,:] ----------------
            pm83 = [t_[:].rearrange("p (two t) -> p two t", two=2) for t_ in pm8]
            ez83 = [t_[:].rearrange("p (two d) -> p two d", two=2) for t_ in ez8]

            # group item order: independent blocks first, the last-finishing
            # e-block's item dead last (its softmax burst is emitted inside
            # the first group, covered by the leading matmuls)
            items = [("f16", e) for e in range(2 * N8, EB)]
            items += [("dr", pr) for pr in range(N8)]
            tail_items = []
            for e in defer:
                it = ("f16", e) if e >= 2 * N8 else ("dr", e // 2)
                if it in items:
                    items.remove(it)
                    tail_items.append(it)
            items += tail_items
            NIT = len(items)

            def c_mm(t, hf, cps, pos):
                kind, v = items[pos]
                kw = dict(start=(pos == 0), stop=(pos == NIT - 1))
                if kind == "f16":
                    nc.tensor.matmul(
                        cps[hf][:],
                        lhsT=pmat[v][:, t * P : (t + 1) * P],
                        rhs=encn[v][:, hf * 512 : (hf + 1) * 512],
                        **kw,
                    )
                else:
                    nc.tensor.matmul(
                        cps[hf][:],
                        lhsT=pm83[v][:, :, t * P : (t + 1) * P],
                        rhs=ez83[v][:, :, hf * 512 : (hf + 1) * 512],
                        perf_mode=DR,
                        **kw,
                    )

            for t in range(EB):
                cps = [
                    psum_s.tile([P, 512], F32, tag="sps", bufs=6, name=f"c{t}_{hf}")
                    for hf in range(2)
                ]
                last = t == EB - 1
                o_t = out_pool.tile([P, D], F32, name="o_t")

                def stage(hf):
                    # pin the PSUM->SBUF copy to DVE (idle in phase C)
                    nc.vector.tensor_copy(
                        out=o_t[:, hf * 512 : (hf + 1) * 512], in_=cps[hf][:]
                    )

                def drain_rows(hf):
                    # row-split drain across both HWDGE queues for a short
                    # tail
                    nc.vector.tensor_copy(
                        out=o_t[:, hf * 512 : (hf + 1) * 512], in_=cps[hf][:]
                    )
                    for pc in range(2):
                        eng = nc.scalar if pc == 0 else nc.sync
                        eng.dma_start(
                            out=out[
                                t * P + pc * 64 : t * P + (pc + 1) * 64,
                                hf * 512 : (hf + 1) * 512,
                            ],
                            in_=o_t[pc * 64 : (pc + 1) * 64, hf * 512 : (hf + 1) * 512],
                        )

                def c_mm_half(cph, pos, c0):
                    # half-width (256-col) accumulation step for hf=1, own
                    # PSUM tile so the two halves are independent groups
                    kind, v = items[pos]
                    kw = dict(start=(pos == 0), stop=(pos == NIT - 1))
                    if kind == "f16":
                        nc.tensor.matmul(
                            cph[:],
                            lhsT=pmat[v][:, t * P : (t + 1) * P],
                            rhs=encn[v][:, 512 + c0 : 512 + c0 + 256],
                            **kw,
                        )
                    else:
                        nc.tensor.matmul(
                            cph[:],
                            lhsT=pm83[v][:, :, t * P : (t + 1) * P],
                            rhs=ez83[v][:, :, 512 + c0 : 512 + c0 + 256],
                            perf_mode=DR,
                            **kw,
                        )

                def drain_half(cph, c0):
                    nc.vector.tensor_copy(
                        out=o_t[:, 512 + c0 : 512 + c0 + 256], in_=cph[:]
                    )
                    for pc in range(2):
                        eng = nc.scalar if pc == 0 else nc.sync
                        eng.dma_start(
                            out=out[
                                t * P + pc * 64 : t * P + (pc + 1) * 64,
                                512 + c0 : 512 + c0 + 256,
                            ],
                            in_=o_t[pc * 64 : (pc + 1) * 64, 512 + c0 : 512 + c0 + 256],
                        )

                if last:
                    # hf-outer for the final block, and the very last bank is
                    # processed as two half-width column groups: the first
                    # half's drain overlaps the second half's matmuls, so only
                    # 128KB of DMA remains after the final matmul
                    for pos in range(NIT):
                        c_mm(t, 0, cps, pos)
                    drain_rows(0)
                    for hk in range(2):
                        cph = psum_s.tile(
                            [P, 256], F32, tag="sps", bufs=6, name=f"ch{hk}"
                        )
                        for pos in range(NIT):
                            c_mm_half(cph, pos, hk * 256)
                        drain_half(cph, hk * 256)
                else:
                    # pos-outer/hf-inner reuses each lhsT across both banks
                    for pos in range(NIT):
                        for hf in range(2):
                            c_mm(t, hf, cps, pos)
                        if t == 0 and pos == 0:
                            # deferred bursts: second-last on DVE, last on
                            # scalar so the two serial chains run in parallel
                            softmax_eb(defer[0])
                            softmax_eb(defer[1], on_scalar=True)
                    for hf in range(2):
                        stage(hf)
                    eng = nc.scalar if t % 2 == 0 else nc.sync
                    eng.dma_start(out=out[t * P : (t + 1) * P, :], in_=o_t[:])

    nc.compile()
    return nc


def _get_nc():
    global _NC_CACHE
    if _NC_CACHE is None:
        _NC_CACHE = _build()
    return _NC_CACHE


def kernel(enc_outputs, dec_outputs, _want_results=False, **_ignored):
    from concourse.bass_utils import run_bass_kernel_spmd

    nc = _get_nc()
    enc_outputs = np.asarray(enc_outputs, dtype=np.float32)
    dec_outputs = np.asarray(dec_outputs, dtype=np.float32)
    in_maps = make_in_maps(enc_outputs, dec_outputs)
    res = run_bass_kernel_spmd(nc, in_maps, core_ids=list(range(B)))
    out = np.stack([res.results[b]["out"] for b in range(B)], axis=0)
    if _want_results:
        return out, res
    return out
